# revision 24
# baseline (speedup 1.0000x reference)
"""Trainium2 Bass kernel for single-head attention returning only the last
query position's context vector.

Reference computation (per batch b):
    q = x[b] @ Wq + bq;  k = x[b] @ Wk + bk;  v = x[b] @ Wv + bv
    scores = q @ k.T / sqrt(D);  w = softmax(scores);  out = (w @ v)[-1]

Only the LAST query row is returned. With weight fusion done on the host
(M2 = Wq @ Wk.T, ub = bq @ Wk.T -- inputs-only preprocessing):
    u     = x[b,-1] @ M2 + ub               [D]
    s     = x[b] @ u                        [S]   (bk.q shift cancels in softmax)
    w     = softmax(s / sqrt(D))                  (scores ~ N(0,1): no max)
    out   = (w @ x[b]) @ Wv + bv            (sum(w) == 1; 1/Z applied at end)

This collapses the O(S*D^2 + S^2*D) attention into two matvec passes over
x[b] plus tiny GEMVs -> the kernel is DMA-bound (~6.1MB/core).

Sharding: data-parallel, one batch element per NeuronCore (B == 8 cores).

Performance structure (from neuron-profile iteration):
  * All wide matmuls use float32r (1 cycle/row when moving dim >= 256 vs 4
    for plain fp32) -- full fp32 data, faster PE feeding mode.
  * DMA triggers are split across both HWDGE-capable engines: ACT issues the
    8 x-tile loads (2 chunks each) while SP issues the weight loads, halving
    the issue ramp that otherwise delays the first bytes.
  * The s-pass (score matvec) is split DVE/GpSimd, and exp+y matmuls are
    pipelined per 4-chunk group so PE work hides under the DMA stream.
  * Single-allocation tiles; DMA issue order queue-nests the u-chain deps.
"""

import numpy as np

import concourse.bass as bass
import concourse.tile as tile
from concourse import bacc, mybir
from concourse.bass_utils import run_bass_kernel_spmd

B, S, D = 8, 2048, 512
P = 128                 # SBUF partitions
NS = S // P             # 16 sequence chunks
ND = D // P             # 4 feature chunks
NG = 4                  # exp/y pipeline groups of 4 chunks
ALPHA = float(1.0 / np.sqrt(D))
N_CORES = 8
DT = mybir.dt.float32
DTR = mybir.dt.float32r
F32 = np.float32
N_DVE = 16              # all s-pass chunks on DVE (Pool lacks TensorScalarPtr)

_CACHE = {}


def build_bass():
    nc = bacc.Bacc("TRN2", target_bir_lowering=False, debug=False,
                   num_devices=N_CORES)

    x_d = nc.dram_tensor("x", [S, D], DT, kind="ExternalInput").ap()
    xlt_d = nc.dram_tensor("xlt", [P, ND], DT, kind="ExternalInput").ap()
    id_d = nc.dram_tensor("ident", [P, P], DT, kind="ExternalInput").ap()
    m2_d = nc.dram_tensor("m2", [D, D], DT, kind="ExternalInput").ap()
    ub_d = nc.dram_tensor("ub", [1, D], DT, kind="ExternalInput").ap()
    wv_d = nc.dram_tensor("wv", [D, D], DT, kind="ExternalInput").ap()
    bv_d = nc.dram_tensor("bv", [1, D], DT, kind="ExternalInput").ap()
    onesr_d = nc.dram_tensor("onesr", [1, P], DT, kind="ExternalInput").ap()
    out_d = nc.dram_tensor("out", [1, D], DT, kind="ExternalOutput").ap()

    mult = mybir.AluOpType.mult
    add = mybir.AluOpType.add
    act_exp = mybir.ActivationFunctionType.Exp

    def r(ap):
        return ap.bitcast(DTR)

    with tile.TileContext(nc) as tc:
        with (
            tc.tile_pool(name="sb", bufs=1) as sb,
            tc.tile_pool(name="ps", bufs=1, space="PSUM") as ps,
        ):
            # ---------------- SBUF tiles (single allocation each) ----------
            xlt = sb.tile([P, ND], DTR, tag="xlt")
            ident = sb.tile([P, P], DT, tag="ident")
            m2_t = sb.tile([P, ND, D], DTR, tag="m2")
            wv_t = sb.tile([P, ND, D], DTR, tag="wv")
            ub_t = sb.tile([1, D], DT, tag="ub")
            bv_t = sb.tile([1, D], DT, tag="bv")
            x_t = sb.tile([P, NS, D], DTR, tag="xall")
            junk = [sb.tile([P, D], DT, tag=f"junk{c}", name=f"junk{c}")
                    for c in range(NS)]

            ones_row = sb.tile([1, P], DTR, tag="ones_row")
            ones_col = sb.tile([P, 1], DT, tag="ones_col")
            u_sb = sb.tile([1, D], DTR, tag="u_sb")
            ubc_sb = sb.tile([P, D], DT, tag="ubc_sb")
            s_all = sb.tile([P, NS], DT, tag="s_all")
            e_all = sb.tile([P, NS], DTR, tag="e_all")
            zz_sb = sb.tile([16, 1], DT, tag="zz_sb")
            rz = sb.tile([1, 1], DT, tag="rz")
            y_sb = sb.tile([1, D], DT, tag="y_sb")
            y_cols = sb.tile([P, ND], DTR, tag="y_cols")
            o_cp = sb.tile([1, D], DT, tag="o_cp")
            o_sb = sb.tile([1, D], DT, tag="o_sb")

            # ---------------- PSUM tiles (7 banks) -------------------------
            u_ps = ps.tile([1, D], DT, tag="u")
            ubc_ps = ps.tile([P, D], DT, tag="ubc")
            zz_ps = ps.tile([16, 1], DT, tag="zz")
            z_ps = ps.tile([1, 1], DT, tag="z")
            y_ps = ps.tile([1, D], DT, tag="y")
            yt4 = ps.tile([P, ND], DT, tag="yt4")
            o_ps = ps.tile([1, D], DT, tag="o")

            # ---------------- DMA issue -----------------------------------
            # ACT issues the 8 x loads (2 chunks each) while SP issues the
            # weights -- parallel trigger ramps. SP order nests xlt before m2
            # so the u matmuls' queue waits cover both.
            for g in range(8):
                nc.scalar.dma_start(
                    out=x_t[:, 2 * g:2 * g + 2, :],
                    in_=x_d[2 * g * P:(2 * g + 2) * P, :].rearrange(
                        "(c p) d -> p c d", p=P).bitcast(DTR))
            # Queue model: 8 HWDGE queues, one in-flight DMA each (a trigger
            # on a reused queue waits for its predecessor's data), ~50GB/s
            # per queue. m2 heads the queues as 8x128KB triggers so the
            # u-chain unblocks ~3us into the stream; wv (only needed by the
            # late o matmuls) is last, split 4 ways.
            dma = nc.sync.dma_start
            for k in range(ND):
                for h in range(2):
                    dma(out=m2_t[:, k, h * 256:(h + 1) * 256],
                        in_=m2_d[k * P:(k + 1) * P,
                                 h * 256:(h + 1) * 256].bitcast(DTR))
            dma(out=xlt[:], in_=xlt_d[:].bitcast(DTR))
            dma(out=ub_t[:], in_=ub_d[:])
            dma(out=ones_row[:], in_=onesr_d[:].bitcast(DTR))
            dma(out=ident[:], in_=id_d[:])
            dma(out=bv_t[:], in_=bv_d[:])
            for k in range(ND):
                dma(out=wv_t[:, k, :],
                    in_=wv_d[k * P:(k + 1) * P, :].bitcast(DTR))

            # ---------------- tiny DVE constants ---------------------------
            nc.vector.memset(ones_col[:], 1.0)

            # ---------------- u = x_last @ M2 + ub        [1, D] -----------
            for k in range(ND):
                nc.tensor.matmul(u_ps[:], lhsT=xlt[:, k:k + 1],
                                 rhs=m2_t[:, k, :],
                                 start=(k == 0), stop=False)
            # + ub via K=1 matmul: ident[0,0] is the constant 1.0
            nc.tensor.matmul(u_ps[:], lhsT=ident[0:1, 0:1], rhs=ub_t[:],
                             start=False, stop=True)
            nc.vector.tensor_copy(u_sb[:], u_ps[:])

            # ---------------- broadcast u across partitions ----------------
            nc.tensor.matmul(ubc_ps[:], lhsT=ones_row[:], rhs=u_sb[:],
                             start=True, stop=True)
            nc.vector.tensor_copy(ubc_sb[:], ubc_ps[:])

            # ---------------- pipelined s -> exp -> y over chunk groups ----
            # s[j] = x[j,:].u on DVE (chunks < N_DVE) / GpSimd (rest);
            # exp per 4-chunk group on ACT; y matmul per chunk on PE.
            for g in range(NG):
                for c in range(4 * g, 4 * g + 4):
                    eng = nc.vector if c < N_DVE else nc.gpsimd
                    eng.scalar_tensor_tensor(
                        out=junk[c][:], in0=x_t[:, c, :].bitcast(DT), scalar=1.0,
                        in1=ubc_sb[:], op0=mult, op1=mult,
                        accum_out=s_all[:, c:c + 1])
                nc.scalar.activation(e_all[:, 4 * g:4 * g + 4],
                                     s_all[:, 4 * g:4 * g + 4],
                                     func=act_exp, scale=ALPHA)
                for c in range(4 * g, 4 * g + 4):
                    nc.tensor.matmul(y_ps[:], lhsT=e_all[:, c:c + 1],
                                     rhs=x_t[:, c, :],
                                     start=(c == 0), stop=(c == NS - 1))

            # ---------------- Z = sum(e); rz = 1/Z -------------------------
            nc.tensor.matmul(zz_ps[:], lhsT=e_all[:].bitcast(DT), rhs=ones_col[:],
                             start=True, stop=True)
            nc.vector.tensor_copy(zz_sb[:], zz_ps[:])
            nc.tensor.matmul(z_ps[:], lhsT=zz_sb[:], rhs=ones_col[0:16, :],
                             start=True, stop=True)
            nc.vector.reciprocal(rz[:], z_ps[:])

            # ---------------- y row -> columns; o = y @ Wv -----------------
            nc.vector.tensor_copy(y_sb[:], y_ps[:])
            for c in range(ND):
                nc.tensor.transpose(yt4[:, c:c + 1], y_sb[0:1, c * P:(c + 1) * P],
                                    ident[0:1, 0:1])
            nc.vector.tensor_copy(y_cols[:], yt4[:])
            for c in range(ND):
                nc.tensor.matmul(o_ps[:], lhsT=y_cols[:, c:c + 1],
                                 rhs=wv_t[:, c, :],
                                 start=(c == 0), stop=(c == ND - 1))

            # ---------------- out = o * (1/Z) + bv -------------------------
            nc.vector.tensor_copy(o_cp[:], o_ps[:])
            nc.vector.scalar_tensor_tensor(
                out=o_sb[:], in0=o_cp[:], scalar=rz[:], in1=bv_t[:],
                op0=mult, op1=add)
            nc.scalar.dma_start(out=out_d[:], in_=o_sb[:])

    nc.compile()
    return nc


def get_bass():
    if "nc" not in _CACHE:
        _CACHE["nc"] = build_bass()
    return _CACHE["nc"]


def make_in_maps(x, Wq, bq, Wk, Wv, bv):
    wq = np.asarray(Wq, dtype=F32)
    wk = np.asarray(Wk, dtype=F32)
    wv = np.ascontiguousarray(Wv, dtype=F32)
    # Host-side weight fusion (inputs-only, independent of x).
    m2 = np.ascontiguousarray(wq @ wk.T)
    ub = np.ascontiguousarray(np.asarray(bq, F32) @ wk.T).reshape(1, D)
    bv2 = np.ascontiguousarray(bv, dtype=F32).reshape(1, D)
    ident = np.eye(P, dtype=F32)
    in_maps = []
    for i in range(N_CORES):
        xb = np.ascontiguousarray(x[i], dtype=F32)
        # x[b, -1, :] laid out as [P, ND] columns: xlt[p, c] = x[b, -1, c*P+p]
        xlt = np.ascontiguousarray(xb[-1].reshape(ND, P).T)
        in_maps.append({"x": xb, "xlt": xlt, "ident": ident, "m2": m2,
                       "ub": ub, "wv": wv, "bv": bv2,
                       "onesr": np.ones((1, P), F32)})
    return in_maps


def kernel(x, Wq, bq, Wk, bk, Wv, bv, **_unused):
    # bk shifts every score by the same bk.q -> cancels in softmax; unused.
    nc = get_bass()
    in_maps = make_in_maps(x, Wq, bq, Wk, Wv, bv)
    res = run_bass_kernel_spmd(nc, in_maps, list(range(N_CORES)))
    out = np.stack([res.results[i]["out"].reshape(D) for i in range(N_CORES)])
    return out.astype(F32)


# revision 25
# speedup vs baseline: 1.1277x; 1.1277x over previous
"""Trainium2 Bass kernel for single-head attention returning only the last
query position's context vector.

Reference computation (per batch b):
    q = x[b] @ Wq + bq;  k = x[b] @ Wk + bk;  v = x[b] @ Wv + bv
    scores = q @ k.T / sqrt(D);  w = softmax(scores);  out = (w @ v)[-1]

Only the LAST query row is returned. With weight fusion done on the host
(M2 = Wq @ Wk.T, ub = bq @ Wk.T -- inputs-only preprocessing):
    u     = x[b,-1] @ M2 + ub               [D]
    s     = x[b] @ u                        [S]   (bk.q shift cancels in softmax)
    w     = softmax(s / sqrt(D))                  (scores ~ N(0,1): no max)
    out   = (w @ x[b]) @ Wv + bv            (sum(w) == 1; 1/Z applied at end)

This collapses the O(S*D^2 + S^2*D) attention into two matvec passes over
x[b] plus tiny GEMVs -> the kernel is DMA-bound (~6.1MB/core).

Sharding: data-parallel, one batch element per NeuronCore (B == 8 cores).

Performance structure (from neuron-profile iteration):
  * All wide matmuls use float32r (1 cycle/row when moving dim >= 256 vs 4
    for plain fp32) -- full fp32 data, faster PE feeding mode.
  * DMA triggers are split across both HWDGE-capable engines: ACT issues the
    8 x-tile loads (2 chunks each) while SP issues the weight loads, halving
    the issue ramp that otherwise delays the first bytes.
  * The s-pass (score matvec) is split DVE/GpSimd, and exp+y matmuls are
    pipelined per 4-chunk group so PE work hides under the DMA stream.
  * Single-allocation tiles; DMA issue order queue-nests the u-chain deps.
"""

import numpy as np

import concourse.bass as bass
import concourse.tile as tile
from concourse import bacc, mybir
from concourse.bass_utils import run_bass_kernel_spmd

B, S, D = 8, 2048, 512
P = 128                 # SBUF partitions
NS = S // P             # 16 sequence chunks
ND = D // P             # 4 feature chunks
NG = 4                  # exp/y pipeline groups of 4 chunks
ALPHA = float(1.0 / np.sqrt(D))
N_CORES = 8
DT = mybir.dt.float32
DTR = mybir.dt.float32r
F32 = np.float32
N_DVE = 16              # all s-pass chunks on DVE (Pool lacks TensorScalarPtr)

_CACHE = {}


def build_bass():
    nc = bacc.Bacc("TRN2", target_bir_lowering=False, debug=False,
                   num_devices=N_CORES)

    x_d = nc.dram_tensor("x", [S, D], DT, kind="ExternalInput").ap()
    xlt_d = nc.dram_tensor("xlt", [P, ND], DT, kind="ExternalInput").ap()
    id_d = nc.dram_tensor("ident", [P, P], DT, kind="ExternalInput").ap()
    m2_d = nc.dram_tensor("m2", [D, D], DT, kind="ExternalInput").ap()
    ub_d = nc.dram_tensor("ub", [1, D], DT, kind="ExternalInput").ap()
    wv_d = nc.dram_tensor("wv", [D, D], DT, kind="ExternalInput").ap()
    bv_d = nc.dram_tensor("bv", [1, D], DT, kind="ExternalInput").ap()
    onesr_d = nc.dram_tensor("onesr", [1, P], DT, kind="ExternalInput").ap()
    out_d = nc.dram_tensor("out", [1, D], DT, kind="ExternalOutput").ap()

    mult = mybir.AluOpType.mult
    add = mybir.AluOpType.add
    act_exp = mybir.ActivationFunctionType.Exp

    def r(ap):
        return ap.bitcast(DTR)

    with tile.TileContext(nc) as tc:
        with (
            tc.tile_pool(name="sb", bufs=1) as sb,
            tc.tile_pool(name="ps", bufs=1, space="PSUM") as ps,
        ):
            # ---------------- SBUF tiles (single allocation each) ----------
            xlt = sb.tile([P, ND], DTR, tag="xlt")
            ident = sb.tile([P, P], DT, tag="ident")
            m2_t = sb.tile([P, ND, D], DTR, tag="m2")
            wv_t = sb.tile([P, ND, D], DTR, tag="wv")
            ub_t = sb.tile([1, D], DT, tag="ub")
            bv_t = sb.tile([1, D], DT, tag="bv")
            x_t = sb.tile([P, NS, D], DTR, tag="xall")
            junk = [sb.tile([P, D], DT, tag=f"junk{c}", name=f"junk{c}")
                    for c in range(NS)]

            ones_row = sb.tile([1, P], DTR, tag="ones_row")
            ones_col = sb.tile([P, 1], DT, tag="ones_col")
            u_sb = sb.tile([1, D], DTR, tag="u_sb")
            ubc_sb = sb.tile([P, D], DT, tag="ubc_sb")
            s_all = sb.tile([P, NS], DT, tag="s_all")
            e_all = sb.tile([P, NS], DTR, tag="e_all")
            zz_sb = sb.tile([16, 1], DT, tag="zz_sb")
            rz = sb.tile([1, 1], DT, tag="rz")
            y_sb = sb.tile([1, D], DT, tag="y_sb")
            y_cols = sb.tile([P, ND], DTR, tag="y_cols")
            o_cp = sb.tile([1, D], DT, tag="o_cp")
            o_sb = sb.tile([1, D], DT, tag="o_sb")

            # ---------------- PSUM tiles (7 banks) -------------------------
            u_ps = ps.tile([1, D], DT, tag="u")
            ubc_ps = ps.tile([P, D], DT, tag="ubc")
            zz_ps = ps.tile([16, 1], DT, tag="zz")
            z_ps = ps.tile([1, 1], DT, tag="z")
            y_ps = ps.tile([1, D], DT, tag="y")
            yt4 = ps.tile([P, ND], DT, tag="yt4")
            o_ps = ps.tile([1, D], DT, tag="o")

            # ---------------- DMA issue -----------------------------------
            # ACT issues the 8 x loads (2 chunks each) while SP issues the
            # weights -- parallel trigger ramps. SP order nests xlt before m2
            # so the u matmuls' queue waits cover both.
            for k in (2, 3):
                for h in range(2):
                    nc.scalar.dma_start(
                        out=m2_t[:, k, h * 256:(h + 1) * 256],
                        in_=m2_d[k * P:(k + 1) * P,
                                 h * 256:(h + 1) * 256].bitcast(DTR))
            # Queue model: 8 HWDGE queues, one in-flight DMA each (a trigger
            # on a reused queue waits for its predecessor's data), ~50GB/s
            # per queue. m2 heads the queues as 8x128KB triggers so the
            # u-chain unblocks ~3us into the stream; wv (only needed by the
            # late o matmuls) is last, split 4 ways.
            dma = nc.sync.dma_start
            for k in (0, 1):
                for h in range(2):
                    dma(out=m2_t[:, k, h * 256:(h + 1) * 256],
                        in_=m2_d[k * P:(k + 1) * P,
                                 h * 256:(h + 1) * 256].bitcast(DTR))
            dma(out=xlt[:], in_=xlt_d[:].bitcast(DTR))
            dma(out=ub_t[:], in_=ub_d[:])
            dma(out=ones_row[:], in_=onesr_d[:].bitcast(DTR))
            dma(out=ident[:], in_=id_d[:])
            dma(out=bv_t[:], in_=bv_d[:])
            for k in range(ND):
                dma(out=wv_t[:, k, :],
                    in_=wv_d[k * P:(k + 1) * P, :].bitcast(DTR))

            # ACT gate: consume one element of every m2 chunk so the x
            # triggers (next in ACT program order) cannot occupy DMA queues
            # until m2 has fully landed -- m2 heads the queues uncontended.
            gate_j = sb.tile([1, 4, 1], DT, tag="gate_j")
            nc.scalar.copy(gate_j[:], m2_t[0:1, :, 0:1].bitcast(DT))
            for g in range(8):
                nc.scalar.dma_start(
                    out=x_t[:, 2 * g:2 * g + 2, :],
                    in_=x_d[2 * g * P:(2 * g + 2) * P, :].rearrange(
                        "(c p) d -> p c d", p=P).bitcast(DTR))

            # ---------------- tiny DVE constants ---------------------------
            nc.vector.memset(ones_col[:], 1.0)

            # ---------------- u = x_last @ M2 + ub        [1, D] -----------
            for k in range(ND):
                nc.tensor.matmul(u_ps[:], lhsT=xlt[:, k:k + 1],
                                 rhs=m2_t[:, k, :],
                                 start=(k == 0), stop=(k == ND - 1))
            # + ub on DVE while copying out of PSUM (keeps ident/ub off the
            # critical u group)
            nc.vector.tensor_add(u_sb[:], u_ps[:], ub_t[:])

            # ---------------- broadcast u across partitions ----------------
            nc.tensor.matmul(ubc_ps[:], lhsT=ones_row[:], rhs=u_sb[:],
                             start=True, stop=True)
            nc.vector.tensor_copy(ubc_sb[:], ubc_ps[:])

            # ---------------- pipelined s -> exp -> y over chunk groups ----
            # s[j] = x[j,:].u on DVE (chunks < N_DVE) / GpSimd (rest);
            # exp per 4-chunk group on ACT; y matmul per chunk on PE.
            for g in range(NG):
                for c in range(4 * g, 4 * g + 4):
                    eng = nc.vector if c < N_DVE else nc.gpsimd
                    eng.scalar_tensor_tensor(
                        out=junk[c][:], in0=x_t[:, c, :].bitcast(DT), scalar=1.0,
                        in1=ubc_sb[:], op0=mult, op1=mult,
                        accum_out=s_all[:, c:c + 1])
                nc.scalar.activation(e_all[:, 4 * g:4 * g + 4],
                                     s_all[:, 4 * g:4 * g + 4],
                                     func=act_exp, scale=ALPHA)
                for c in range(4 * g, 4 * g + 4):
                    nc.tensor.matmul(y_ps[:], lhsT=e_all[:, c:c + 1],
                                     rhs=x_t[:, c, :],
                                     start=(c == 0), stop=(c == NS - 1))

            # ---------------- Z = sum(e); rz = 1/Z -------------------------
            nc.tensor.matmul(zz_ps[:], lhsT=e_all[:].bitcast(DT), rhs=ones_col[:],
                             start=True, stop=True)
            nc.vector.tensor_copy(zz_sb[:], zz_ps[:])
            nc.tensor.matmul(z_ps[:], lhsT=zz_sb[:], rhs=ones_col[0:16, :],
                             start=True, stop=True)
            nc.vector.reciprocal(rz[:], z_ps[:])

            # ---------------- y row -> columns; o = y @ Wv -----------------
            nc.vector.tensor_copy(y_sb[:], y_ps[:])
            for c in range(ND):
                nc.tensor.transpose(yt4[:, c:c + 1], y_sb[0:1, c * P:(c + 1) * P],
                                    ident[0:1, 0:1])
            nc.vector.tensor_copy(y_cols[:], yt4[:])
            for c in range(ND):
                nc.tensor.matmul(o_ps[:], lhsT=y_cols[:, c:c + 1],
                                 rhs=wv_t[:, c, :],
                                 start=(c == 0), stop=(c == ND - 1))

            # ---------------- out = o * (1/Z) + bv -------------------------
            nc.vector.tensor_copy(o_cp[:], o_ps[:])
            nc.vector.scalar_tensor_tensor(
                out=o_sb[:], in0=o_cp[:], scalar=rz[:], in1=bv_t[:],
                op0=mult, op1=add)
            nc.scalar.dma_start(out=out_d[:], in_=o_sb[:])

    nc.compile()
    return nc


def get_bass():
    if "nc" not in _CACHE:
        _CACHE["nc"] = build_bass()
    return _CACHE["nc"]


def make_in_maps(x, Wq, bq, Wk, Wv, bv):
    wq = np.asarray(Wq, dtype=F32)
    wk = np.asarray(Wk, dtype=F32)
    wv = np.ascontiguousarray(Wv, dtype=F32)
    # Host-side weight fusion (inputs-only, independent of x).
    m2 = np.ascontiguousarray(wq @ wk.T)
    ub = np.ascontiguousarray(np.asarray(bq, F32) @ wk.T).reshape(1, D)
    bv2 = np.ascontiguousarray(bv, dtype=F32).reshape(1, D)
    ident = np.eye(P, dtype=F32)
    in_maps = []
    for i in range(N_CORES):
        xb = np.ascontiguousarray(x[i], dtype=F32)
        # x[b, -1, :] laid out as [P, ND] columns: xlt[p, c] = x[b, -1, c*P+p]
        xlt = np.ascontiguousarray(xb[-1].reshape(ND, P).T)
        in_maps.append({"x": xb, "xlt": xlt, "ident": ident, "m2": m2,
                       "ub": ub, "wv": wv, "bv": bv2,
                       "onesr": np.ones((1, P), F32)})
    return in_maps


def kernel(x, Wq, bq, Wk, bk, Wv, bv, **_unused):
    # bk shifts every score by the same bk.q -> cancels in softmax; unused.
    nc = get_bass()
    in_maps = make_in_maps(x, Wq, bq, Wk, Wv, bv)
    res = run_bass_kernel_spmd(nc, in_maps, list(range(N_CORES)))
    out = np.stack([res.results[i]["out"].reshape(D) for i in range(N_CORES)])
    return out.astype(F32)


# revision 26
# speedup vs baseline: 1.2126x; 1.0753x over previous
"""Trainium2 Bass kernel for single-head attention returning only the last
query position's context vector.

Reference computation (per batch b):
    q = x[b] @ Wq + bq;  k = x[b] @ Wk + bk;  v = x[b] @ Wv + bv
    scores = q @ k.T / sqrt(D);  w = softmax(scores);  out = (w @ v)[-1]

Only the LAST query row is returned. With weight fusion done on the host
(M2 = Wq @ Wk.T, ub = bq @ Wk.T -- inputs-only preprocessing):
    u     = x[b,-1] @ M2 + ub               [D]
    s     = x[b] @ u                        [S]   (bk.q shift cancels in softmax)
    w     = softmax(s / sqrt(D))                  (scores ~ N(0,1): no max)
    out   = (w @ x[b]) @ Wv + bv            (sum(w) == 1; 1/Z applied at end)

This collapses the O(S*D^2 + S^2*D) attention into two matvec passes over
x[b] plus tiny GEMVs -> the kernel is DMA-bound (~6.1MB/core).

Sharding: data-parallel, one batch element per NeuronCore (B == 8 cores).

Performance structure (from neuron-profile iteration):
  * All wide matmuls use float32r (1 cycle/row when moving dim >= 256 vs 4
    for plain fp32) -- full fp32 data, faster PE feeding mode.
  * DMA triggers are split across both HWDGE-capable engines: ACT issues the
    8 x-tile loads (2 chunks each) while SP issues the weight loads, halving
    the issue ramp that otherwise delays the first bytes.
  * The s-pass (score matvec) is split DVE/GpSimd, and exp+y matmuls are
    pipelined per 4-chunk group so PE work hides under the DMA stream.
  * Single-allocation tiles; DMA issue order queue-nests the u-chain deps.
"""

import numpy as np

import concourse.bass as bass
import concourse.tile as tile
from concourse import bacc, mybir
from concourse.bass_utils import run_bass_kernel_spmd

B, S, D = 8, 2048, 512
P = 128                 # SBUF partitions
NS = S // P             # 16 sequence chunks
ND = D // P             # 4 feature chunks
NG = 4                  # exp/y pipeline groups of 4 chunks
ALPHA = float(1.0 / np.sqrt(D))
N_CORES = 8
DT = mybir.dt.float32
DTR = mybir.dt.float32r
F32 = np.float32
N_DVE = 16              # all s-pass chunks on DVE (Pool lacks TensorScalarPtr)

_CACHE = {}


def build_bass():
    nc = bacc.Bacc("TRN2", target_bir_lowering=False, debug=False,
                   num_devices=N_CORES)

    x_d = nc.dram_tensor("x", [S, D], DT, kind="ExternalInput").ap()
    xlt_d = nc.dram_tensor("xlt", [P, ND], DT, kind="ExternalInput").ap()
    id_d = nc.dram_tensor("ident", [P, P], DT, kind="ExternalInput").ap()
    m2_d = nc.dram_tensor("m2", [D, D], DT, kind="ExternalInput").ap()
    ub_d = nc.dram_tensor("ub", [1, D], DT, kind="ExternalInput").ap()
    wv_d = nc.dram_tensor("wv", [D, D], DT, kind="ExternalInput").ap()
    bv_d = nc.dram_tensor("bv", [1, D], DT, kind="ExternalInput").ap()
    onesr_d = nc.dram_tensor("onesr", [1, P], DT, kind="ExternalInput").ap()
    out_d = nc.dram_tensor("out", [1, D], DT, kind="ExternalOutput").ap()

    mult = mybir.AluOpType.mult
    add = mybir.AluOpType.add
    act_exp = mybir.ActivationFunctionType.Exp

    def r(ap):
        return ap.bitcast(DTR)

    with tile.TileContext(nc) as tc:
        with (
            tc.tile_pool(name="sb", bufs=1) as sb,
            tc.tile_pool(name="ps", bufs=1, space="PSUM") as ps,
        ):
            # ---------------- SBUF tiles (single allocation each) ----------
            xlt = sb.tile([P, ND], DTR, tag="xlt")
            ident = sb.tile([P, P], DT, tag="ident")
            m2_t = sb.tile([P, ND, D], DTR, tag="m2")
            wv_t = sb.tile([P, ND, D], DTR, tag="wv")
            ub_t = sb.tile([1, D], DT, tag="ub")
            bv_t = sb.tile([1, D], DT, tag="bv")
            x_t = sb.tile([P, NS, D], DTR, tag="xall")
            junk = [sb.tile([P, D], DT, tag=f"junk{c}", name=f"junk{c}")
                    for c in range(NS)]

            ones_row = sb.tile([1, P], DTR, tag="ones_row")
            ones_col = sb.tile([P, 1], DT, tag="ones_col")
            u_sb = sb.tile([1, D], DTR, tag="u_sb")
            ubc_sb = sb.tile([P, D], DT, tag="ubc_sb")
            s_all = sb.tile([P, NS], DT, tag="s_all")
            e_all = sb.tile([P, NS], DTR, tag="e_all")
            zz_sb = sb.tile([16, 1], DT, tag="zz_sb")
            rz = sb.tile([1, 1], DT, tag="rz")
            y_sb = sb.tile([1, D], DT, tag="y_sb")
            y_cols = sb.tile([P, ND], DTR, tag="y_cols")
            o_sb = sb.tile([1, D], DT, tag="o_sb")

            # ---------------- PSUM tiles (7 banks) -------------------------
            u_ps = ps.tile([1, D], DT, tag="u")
            ubc_ps = ps.tile([P, D], DT, tag="ubc")
            zz_ps = ps.tile([16, 1], DT, tag="zz")
            z_ps = ps.tile([1, 1], DT, tag="z")
            y_ps = ps.tile([1, D], DT, tag="y")
            yt4 = ps.tile([P, ND], DT, tag="yt4")
            o_ps = ps.tile([1, D], DT, tag="o")

            # ---------------- DMA issue -----------------------------------
            # ACT issues the 8 x loads (2 chunks each) while SP issues the
            # weights -- parallel trigger ramps. SP order nests xlt before m2
            # so the u matmuls' queue waits cover both.
            for k in (2, 3):
                for h in range(2):
                    nc.scalar.dma_start(
                        out=m2_t[:, k, h * 256:(h + 1) * 256],
                        in_=m2_d[k * P:(k + 1) * P,
                                 h * 256:(h + 1) * 256].bitcast(DTR))
            # Queue model: 8 HWDGE queues, one in-flight DMA each (a trigger
            # on a reused queue waits for its predecessor's data), ~50GB/s
            # per queue. m2 heads the queues as 8x128KB triggers so the
            # u-chain unblocks ~3us into the stream; wv (only needed by the
            # late o matmuls) is last, split 4 ways.
            dma = nc.sync.dma_start
            for k in (0, 1):
                for h in range(2):
                    dma(out=m2_t[:, k, h * 256:(h + 1) * 256],
                        in_=m2_d[k * P:(k + 1) * P,
                                 h * 256:(h + 1) * 256].bitcast(DTR))
            dma(out=xlt[:], in_=xlt_d[:].bitcast(DTR))
            dma(out=ub_t[:], in_=ub_d[:])
            dma(out=ones_row[:], in_=onesr_d[:].bitcast(DTR))
            dma(out=ident[:], in_=id_d[:])
            dma(out=bv_t[:], in_=bv_d[:])
            for k in range(ND):
                dma(out=wv_t[:, k, :],
                    in_=wv_d[k * P:(k + 1) * P, :].bitcast(DTR))

            # ACT gate: consume one element of every m2 chunk so the x
            # triggers (next in ACT program order) cannot occupy DMA queues
            # until m2 has fully landed -- m2 heads the queues uncontended.
            gate_j = sb.tile([1, 4, 1], DT, tag="gate_j")
            nc.scalar.copy(gate_j[:], m2_t[0:1, :, 0:1].bitcast(DT))
            for g in range(6):
                nc.scalar.dma_start(
                    out=x_t[:, 2 * g:2 * g + 2, :],
                    in_=x_d[2 * g * P:(2 * g + 2) * P, :].rearrange(
                        "(c p) d -> p c d", p=P).bitcast(DTR))
            for c in range(12, NS):
                nc.scalar.dma_start(
                    out=x_t[:, c:c + 1, :],
                    in_=x_d[c * P:(c + 1) * P, :].rearrange(
                        "(c p) d -> p c d", p=P).bitcast(DTR))

            # ---------------- tiny DVE constants ---------------------------
            nc.vector.memset(ones_col[:], 1.0)

            # ---------------- u = x_last @ M2 + ub        [1, D] -----------
            for k in range(ND):
                nc.tensor.matmul(u_ps[:], lhsT=xlt[:, k:k + 1],
                                 rhs=m2_t[:, k, :],
                                 start=(k == 0), stop=(k == ND - 1))
            # + ub on DVE while copying out of PSUM (keeps ident/ub off the
            # critical u group)
            nc.vector.tensor_add(u_sb[:], u_ps[:], ub_t[:])

            # ---------------- broadcast u across partitions ----------------
            nc.tensor.matmul(ubc_ps[:], lhsT=ones_row[:], rhs=u_sb[:],
                             start=True, stop=True)
            nc.vector.tensor_copy(ubc_sb[:], ubc_ps[:])

            # ---------------- pipelined s -> exp -> y over chunk groups ----
            # s[j] = x[j,:].u on DVE (chunks < N_DVE) / GpSimd (rest);
            # exp per 4-chunk group on ACT; y matmul per chunk on PE.
            groups = [(0, 4), (4, 8), (8, 10), (10, 12), (12, 14), (14, 16)]
            for lo, hi in groups:
                for c in range(lo, hi):
                    nc.vector.scalar_tensor_tensor(
                        out=junk[c][:], in0=x_t[:, c, :].bitcast(DT), scalar=1.0,
                        in1=ubc_sb[:], op0=mult, op1=mult,
                        accum_out=s_all[:, c:c + 1])
                nc.scalar.activation(e_all[:, lo:hi], s_all[:, lo:hi],
                                     func=act_exp, scale=ALPHA)
                for c in range(lo, hi):
                    nc.tensor.matmul(y_ps[:], lhsT=e_all[:, c:c + 1],
                                     rhs=x_t[:, c, :],
                                     start=(c == 0), stop=(c == NS - 1))

            # ---------------- Z = sum(e); rz = 1/Z -------------------------
            nc.tensor.matmul(zz_ps[:], lhsT=e_all[:].bitcast(DT), rhs=ones_col[:],
                             start=True, stop=True)
            nc.vector.tensor_copy(zz_sb[:], zz_ps[:])
            nc.tensor.matmul(z_ps[:], lhsT=zz_sb[:], rhs=ones_col[0:16, :],
                             start=True, stop=True)
            nc.vector.reciprocal(rz[:], z_ps[:])

            # ---------------- y row -> columns; o = y @ Wv -----------------
            nc.vector.tensor_copy(y_sb[:], y_ps[:])
            for c in range(ND):
                nc.tensor.transpose(yt4[:, c:c + 1], y_sb[0:1, c * P:(c + 1) * P],
                                    ident[0:1, 0:1])
            nc.vector.tensor_copy(y_cols[:], yt4[:])
            for c in range(ND):
                nc.tensor.matmul(o_ps[:], lhsT=y_cols[:, c:c + 1],
                                 rhs=wv_t[:, c, :],
                                 start=(c == 0), stop=(c == ND - 1))

            # ---------------- out = o * (1/Z) + bv -------------------------
            nc.vector.scalar_tensor_tensor(
                out=o_sb[:], in0=o_ps[:], scalar=rz[:], in1=bv_t[:],
                op0=mult, op1=add)
            nc.scalar.dma_start(out=out_d[:], in_=o_sb[:])

    nc.compile()
    return nc


def get_bass():
    if "nc" not in _CACHE:
        _CACHE["nc"] = build_bass()
    return _CACHE["nc"]


def make_in_maps(x, Wq, bq, Wk, Wv, bv):
    wq = np.asarray(Wq, dtype=F32)
    wk = np.asarray(Wk, dtype=F32)
    wv = np.ascontiguousarray(Wv, dtype=F32)
    # Host-side weight fusion (inputs-only, independent of x).
    m2 = np.ascontiguousarray(wq @ wk.T)
    ub = np.ascontiguousarray(np.asarray(bq, F32) @ wk.T).reshape(1, D)
    bv2 = np.ascontiguousarray(bv, dtype=F32).reshape(1, D)
    ident = np.eye(P, dtype=F32)
    in_maps = []
    for i in range(N_CORES):
        xb = np.ascontiguousarray(x[i], dtype=F32)
        # x[b, -1, :] laid out as [P, ND] columns: xlt[p, c] = x[b, -1, c*P+p]
        xlt = np.ascontiguousarray(xb[-1].reshape(ND, P).T)
        in_maps.append({"x": xb, "xlt": xlt, "ident": ident, "m2": m2,
                       "ub": ub, "wv": wv, "bv": bv2,
                       "onesr": np.ones((1, P), F32)})
    return in_maps


def kernel(x, Wq, bq, Wk, bk, Wv, bv, **_unused):
    # bk shifts every score by the same bk.q -> cancels in softmax; unused.
    nc = get_bass()
    in_maps = make_in_maps(x, Wq, bq, Wk, Wv, bv)
    res = run_bass_kernel_spmd(nc, in_maps, list(range(N_CORES)))
    out = np.stack([res.results[i]["out"].reshape(D) for i in range(N_CORES)])
    return out.astype(F32)


# revision 27
# speedup vs baseline: 1.2709x; 1.0481x over previous
"""Trainium2 Bass kernel for single-head attention returning only the last
query position's context vector.

Reference computation (per batch b):
    q = x[b] @ Wq + bq;  k = x[b] @ Wk + bk;  v = x[b] @ Wv + bv
    scores = q @ k.T / sqrt(D);  w = softmax(scores);  out = (w @ v)[-1]

Only the LAST query row is returned. With weight fusion done on the host
(M2 = Wq @ Wk.T, ub = bq @ Wk.T -- inputs-only preprocessing):
    u     = x[b,-1] @ M2 + ub               [D]
    s     = x[b] @ u                        [S]   (bk.q shift cancels in softmax)
    w     = softmax(s / sqrt(D))                  (scores ~ N(0,1): no max)
    out   = (w @ x[b]) @ Wv + bv            (sum(w) == 1; 1/Z applied at end)

This collapses the O(S*D^2 + S^2*D) attention into two matvec passes over
x[b] plus tiny GEMVs -> the kernel is DMA-bound (~6.1MB/core).

Sharding: data-parallel, one batch element per NeuronCore (B == 8 cores).

Performance structure (from neuron-profile iteration):
  * All wide matmuls use float32r (1 cycle/row when moving dim >= 256 vs 4
    for plain fp32) -- full fp32 data, faster PE feeding mode.
  * DMA triggers are split across both HWDGE-capable engines: ACT issues the
    8 x-tile loads (2 chunks each) while SP issues the weight loads, halving
    the issue ramp that otherwise delays the first bytes.
  * The s-pass (score matvec) is split DVE/GpSimd, and exp+y matmuls are
    pipelined per 4-chunk group so PE work hides under the DMA stream.
  * Single-allocation tiles; DMA issue order queue-nests the u-chain deps.
"""

import ml_dtypes
import numpy as np

import concourse.bass as bass
import concourse.tile as tile
from concourse import bacc, mybir
from concourse.bass_utils import run_bass_kernel_spmd

B, S, D = 8, 2048, 512
P = 128                 # SBUF partitions
NS = S // P             # 16 sequence chunks
ND = D // P             # 4 feature chunks
NG = 4                  # exp/y pipeline groups of 4 chunks
ALPHA = float(1.0 / np.sqrt(D))
N_CORES = 8
DT = mybir.dt.float32
DTR = mybir.dt.float32r
BF16 = mybir.dt.bfloat16
F32 = np.float32
N_DVE = 16              # all s-pass chunks on DVE (Pool lacks TensorScalarPtr)

_CACHE = {}


def build_bass():
    nc = bacc.Bacc("TRN2", target_bir_lowering=False, debug=False,
                   num_devices=N_CORES)

    x_d = nc.dram_tensor("x", [S, D], BF16, kind="ExternalInput").ap()
    xlt_d = nc.dram_tensor("xlt", [P, ND], DT, kind="ExternalInput").ap()
    id_d = nc.dram_tensor("ident", [P, P], DT, kind="ExternalInput").ap()
    m2_d = nc.dram_tensor("m2", [D, D], DT, kind="ExternalInput").ap()
    ub_d = nc.dram_tensor("ub", [1, D], DT, kind="ExternalInput").ap()
    wv_d = nc.dram_tensor("wv", [D, D], DT, kind="ExternalInput").ap()
    bv_d = nc.dram_tensor("bv", [1, D], DT, kind="ExternalInput").ap()
    onesr_d = nc.dram_tensor("onesr", [1, P], DT, kind="ExternalInput").ap()
    out_d = nc.dram_tensor("out", [1, D], DT, kind="ExternalOutput").ap()

    mult = mybir.AluOpType.mult
    add = mybir.AluOpType.add
    act_exp = mybir.ActivationFunctionType.Exp

    def r(ap):
        return ap.bitcast(DTR)

    with tile.TileContext(nc) as tc:
        with (
            tc.tile_pool(name="sb", bufs=1) as sb,
            tc.tile_pool(name="ps", bufs=1, space="PSUM") as ps,
        ):
            # ---------------- SBUF tiles (single allocation each) ----------
            xlt = sb.tile([P, ND], DTR, tag="xlt")
            ident = sb.tile([P, P], DT, tag="ident")
            m2_t = sb.tile([P, ND, D], DTR, tag="m2")
            wv_t = sb.tile([P, ND, D], DTR, tag="wv")
            ub_t = sb.tile([1, D], DT, tag="ub")
            bv_t = sb.tile([1, D], DT, tag="bv")
            x_t = sb.tile([P, NS, D], BF16, tag="xall")
            junk = [sb.tile([P, D], BF16, tag=f"junk{c}", name=f"junk{c}")
                    for c in range(NS)]

            ones_row = sb.tile([1, P], DTR, tag="ones_row")
            ones_col = sb.tile([P, 1], DT, tag="ones_col")
            ones_col_b = sb.tile([P, 1], BF16, tag="ones_col_b")
            u_sb = sb.tile([1, D], DTR, tag="u_sb")
            ubc_sb = sb.tile([P, D], BF16, tag="ubc_sb")
            s_all = sb.tile([P, NS], DT, tag="s_all")
            e_all = sb.tile([P, NS], BF16, tag="e_all")
            zz_sb = sb.tile([16, 1], DT, tag="zz_sb")
            rz = sb.tile([1, 1], DT, tag="rz")
            y_sb = sb.tile([1, D], DT, tag="y_sb")
            y_cols = sb.tile([P, ND], DTR, tag="y_cols")
            o_sb = sb.tile([1, D], DT, tag="o_sb")

            # ---------------- PSUM tiles (7 banks) -------------------------
            u_ps = ps.tile([1, D], DT, tag="u")
            ubc_ps = ps.tile([P, D], DT, tag="ubc")
            zz_ps = ps.tile([16, 1], DT, tag="zz")
            z_ps = ps.tile([1, 1], DT, tag="z")
            y_ps = ps.tile([1, D], DT, tag="y")
            yt4 = ps.tile([P, ND], DT, tag="yt4")
            o_ps = ps.tile([1, D], DT, tag="o")

            # ---------------- DMA issue -----------------------------------
            # ACT issues the 8 x loads (2 chunks each) while SP issues the
            # weights -- parallel trigger ramps. SP order nests xlt before m2
            # so the u matmuls' queue waits cover both.
            for k in (2, 3):
                for h in range(2):
                    nc.scalar.dma_start(
                        out=m2_t[:, k, h * 256:(h + 1) * 256],
                        in_=m2_d[k * P:(k + 1) * P,
                                 h * 256:(h + 1) * 256].bitcast(DTR))
            # Queue model: 8 HWDGE queues, one in-flight DMA each (a trigger
            # on a reused queue waits for its predecessor's data), ~50GB/s
            # per queue. m2 heads the queues as 8x128KB triggers so the
            # u-chain unblocks ~3us into the stream; wv (only needed by the
            # late o matmuls) is last, split 4 ways.
            dma = nc.sync.dma_start
            for k in (0, 1):
                for h in range(2):
                    dma(out=m2_t[:, k, h * 256:(h + 1) * 256],
                        in_=m2_d[k * P:(k + 1) * P,
                                 h * 256:(h + 1) * 256].bitcast(DTR))
            dma(out=xlt[:], in_=xlt_d[:].bitcast(DTR))
            dma(out=ub_t[:], in_=ub_d[:])
            dma(out=ones_row[:], in_=onesr_d[:].bitcast(DTR))
            dma(out=ident[:], in_=id_d[:])
            dma(out=bv_t[:], in_=bv_d[:])
            for k in range(ND):
                dma(out=wv_t[:, k, :],
                    in_=wv_d[k * P:(k + 1) * P, :].bitcast(DTR))

            # ACT gate: consume one element of every m2 chunk so the x
            # triggers (next in ACT program order) cannot occupy DMA queues
            # until m2 has fully landed -- m2 heads the queues uncontended.
            gate_j = sb.tile([1, 4, 1], DT, tag="gate_j")
            nc.scalar.copy(gate_j[:], m2_t[0:1, :, 0:1].bitcast(DT))
            for g in range(6):
                nc.scalar.dma_start(
                    out=x_t[:, 2 * g:2 * g + 2, :],
                    in_=x_d[2 * g * P:(2 * g + 2) * P, :].rearrange(
                        "(c p) d -> p c d", p=P))
            for c in range(12, NS):
                nc.scalar.dma_start(
                    out=x_t[:, c:c + 1, :],
                    in_=x_d[c * P:(c + 1) * P, :].rearrange(
                        "(c p) d -> p c d", p=P))

            # ---------------- tiny DVE constants ---------------------------
            nc.vector.memset(ones_col[:], 1.0)
            nc.vector.memset(ones_col_b[:], 1.0)

            # ---------------- u = x_last @ M2 + ub        [1, D] -----------
            for k in range(ND):
                nc.tensor.matmul(u_ps[:], lhsT=xlt[:, k:k + 1],
                                 rhs=m2_t[:, k, :],
                                 start=(k == 0), stop=(k == ND - 1))
            # + ub on DVE while copying out of PSUM (keeps ident/ub off the
            # critical u group)
            nc.vector.tensor_add(u_sb[:], u_ps[:], ub_t[:])

            # ---------------- broadcast u across partitions ----------------
            nc.tensor.matmul(ubc_ps[:], lhsT=ones_row[:], rhs=u_sb[:],
                             start=True, stop=True)
            nc.vector.tensor_copy(ubc_sb[:], ubc_ps[:])

            # ---------------- pipelined s -> exp -> y over chunk groups ----
            # s[j] = x[j,:].u on DVE (chunks < N_DVE) / GpSimd (rest);
            # exp per 4-chunk group on ACT; y matmul per chunk on PE.
            groups = [(0, 4), (4, 8), (8, 10), (10, 12), (12, 14), (14, 16)]
            for lo, hi in groups:
                for c in range(lo, hi):
                    nc.vector.scalar_tensor_tensor(
                        out=junk[c][:], in0=x_t[:, c, :], scalar=1.0,
                        in1=ubc_sb[:], op0=mult, op1=mult,
                        accum_out=s_all[:, c:c + 1])
                nc.scalar.activation(e_all[:, lo:hi], s_all[:, lo:hi],
                                     func=act_exp, scale=ALPHA)
                for c in range(lo, hi):
                    nc.tensor.matmul(y_ps[:], lhsT=e_all[:, c:c + 1],
                                     rhs=x_t[:, c, :],
                                     start=(c == 0), stop=(c == NS - 1))

            # ---------------- Z = sum(e); rz = 1/Z -------------------------
            nc.tensor.matmul(zz_ps[:], lhsT=e_all[:], rhs=ones_col_b[:],
                             start=True, stop=True)
            nc.vector.tensor_copy(zz_sb[:], zz_ps[:])
            nc.tensor.matmul(z_ps[:], lhsT=zz_sb[:], rhs=ones_col[0:16, :],
                             start=True, stop=True)
            nc.vector.reciprocal(rz[:], z_ps[:])

            # ---------------- y row -> columns; o = y @ Wv -----------------
            nc.vector.tensor_copy(y_sb[:], y_ps[:])
            for c in range(ND):
                nc.tensor.transpose(yt4[:, c:c + 1], y_sb[0:1, c * P:(c + 1) * P],
                                    ident[0:1, 0:1])
            nc.vector.tensor_copy(y_cols[:], yt4[:])
            for c in range(ND):
                nc.tensor.matmul(o_ps[:], lhsT=y_cols[:, c:c + 1],
                                 rhs=wv_t[:, c, :],
                                 start=(c == 0), stop=(c == ND - 1))

            # ---------------- out = o * (1/Z) + bv -------------------------
            nc.vector.scalar_tensor_tensor(
                out=o_sb[:], in0=o_ps[:], scalar=rz[:], in1=bv_t[:],
                op0=mult, op1=add)
            nc.scalar.dma_start(out=out_d[:], in_=o_sb[:])

    nc.compile()
    return nc


def get_bass():
    if "nc" not in _CACHE:
        _CACHE["nc"] = build_bass()
    return _CACHE["nc"]


def make_in_maps(x, Wq, bq, Wk, Wv, bv):
    wq = np.asarray(Wq, dtype=F32)
    wk = np.asarray(Wk, dtype=F32)
    wv = np.ascontiguousarray(Wv, dtype=F32)
    # Host-side weight fusion (inputs-only, independent of x).
    m2 = np.ascontiguousarray(wq @ wk.T)
    ub = np.ascontiguousarray(np.asarray(bq, F32) @ wk.T).reshape(1, D)
    bv2 = np.ascontiguousarray(bv, dtype=F32).reshape(1, D)
    ident = np.eye(P, dtype=F32)
    in_maps = []
    for i in range(N_CORES):
        xb = np.ascontiguousarray(x[i], dtype=F32)
        xb16 = xb.astype(ml_dtypes.bfloat16)
        # x[b, -1, :] laid out as [P, ND] columns: xlt[p, c] = x[b, -1, c*P+p]
        # (kept fp32: the u chain stays full precision)
        xlt = np.ascontiguousarray(xb[-1].reshape(ND, P).T)
        in_maps.append({"x": xb16, "xlt": xlt, "ident": ident, "m2": m2,
                       "ub": ub, "wv": wv, "bv": bv2,
                       "onesr": np.ones((1, P), F32)})
    return in_maps


def kernel(x, Wq, bq, Wk, bk, Wv, bv, **_unused):
    # bk shifts every score by the same bk.q -> cancels in softmax; unused.
    nc = get_bass()
    in_maps = make_in_maps(x, Wq, bq, Wk, Wv, bv)
    res = run_bass_kernel_spmd(nc, in_maps, list(range(N_CORES)))
    out = np.stack([res.results[i]["out"].reshape(D) for i in range(N_CORES)])
    return out.astype(F32)
